# revision 21
# baseline (speedup 1.0000x reference)
"""Trainium2 Bass kernel for nn_DensityLoss (retrieval kNN hinge loss).

Computes mean(relu(topk_smallest_dist(x_pred, x_target, k) - 1.0)).

Strategy (8 NeuronCores, SPMD, x_pred rows sharded; 1024 rows/core):
  Every one of the 16384 dot products per row must be drained out of PSUM
  by exactly one compute-engine read (DMA and Pool cannot touch PSUM).
  The drain is split between the DVE and ScalarE, each with its own
  2-deep PSUM tile rotation so both stream independently:

  - V region (positions    0.. 8191): DVE tensor_reduce(axis=X) over a
    [128, 64, 16] PSUM view drains AND folds a 1024-group in one op ->
    512 width-16 chunk maxima per row.
  - W region (positions 8192..16383): ScalarE evacuates fp32->fp16 and
    the slab is shipped raw -> 8192 width-1 near-exact scores per row.

  The kernel runs group-PAIR-major (outer: 8 (V,W) group pairs; inner:
  8 rowtiles), so compute starts as soon as the first b_t slices land
  instead of waiting for the whole 4 MiB load, and the TensorE stays
  continuously busy (p-state).

  Host: targets are b2-sorted (pure sort, no permutation needed: chunk =
  16 consecutive ranks). Chunk score = chunk-max(2ab) - min b2; top-CV
  V-chunks + top-KW W-columns are rescored exactly in float64 -> top-k
  -> hinge -> mean.
"""

import numpy as np

N_CORES = 8
N_PRED = 8192
N_TGT = 16384
DIM = 128
ROWS_PER_CORE = N_PRED // N_CORES  # 1024
ROWTILES = ROWS_PER_CORE // 128    # 8
BANK = 512                         # fp32 PSUM bank, matmul max N
GRP = 1024                         # drain group width (2 PSUM banks)
NPAIR = 8                          # (V, W) group pairs per rowtile
WIN = 16                           # V-chunk width
VQ = 768                           # of the last V-group, DVE reduces
                                   # [0:VQ]; ScalarE evacs [VQ:1024]
                                   # (engine balance: DVE is the pacer)
V_END = 7 * GRP + VQ               # V region positions [0, 7936)
NVCH = V_END // WIN                # 496 V-chunks
NW = 8192 + (GRP - VQ)             # 8448 width-1 raw columns
CV = 12                            # top chunks, V family (width 16)
KW = 16                            # raw width-1 candidates
HINGE = 1.0

_CACHE = {}


def _build_nc():
    import concourse.bacc as bacc
    import concourse.bass as bass
    import concourse.mybir as mybir
    import concourse.tile as tile

    dt = mybir.dt
    nc = bacc.Bacc(
        "TRN2",
        target_bir_lowering=False,
        debug=False,
        num_devices=N_CORES,
    )
    a_t = nc.dram_tensor("a_t", [DIM, ROWS_PER_CORE], dt.bfloat16, kind="ExternalInput")
    b_t = nc.dram_tensor("b_t", [DIM, N_TGT], dt.bfloat16, kind="ExternalInput")
    vout = nc.dram_tensor(
        "vout", [ROWTILES, 128, NVCH], dt.float16, kind="ExternalOutput"
    )
    wraw = nc.dram_tensor(
        "wraw", [ROWTILES, 128, NW], dt.float16, kind="ExternalOutput"
    )

    AX = mybir.AxisListType.X
    MAX = mybir.AluOpType.max

    with tile.TileContext(nc) as tc:
        with (
            tc.tile_pool(name="const", bufs=1) as cpool,
            tc.tile_pool(name="vp", bufs=2, space="PSUM") as vpp,
            tc.tile_pool(name="sp", bufs=2, space="PSUM") as spp,
            tc.tile_pool(name="slab", bufs=4) as spool,
        ):
            bt_sb = cpool.tile([DIM, N_TGT], dt.bfloat16)
            at_sb = cpool.tile([DIM, ROWS_PER_CORE], dt.bfloat16)
            # per-rowtile V-chunk accumulators, DMA'd out at the end
            vt_sb = cpool.tile([128, ROWTILES, NVCH], dt.float16)

            # Startup-critical inputs issue from TWO idle sequencers in
            # parallel (each HWDGE issue costs ~600-670 ns; serializing
            # them on sync alone delays the first matmuls by ~2-3 us):
            # sync: a_t, scalar: pair-0's two b_t slices, then sync
            # streams the rest in first-use order. (Only SP/Activation
            # can initiate HWDGE DMAs.)
            nc.sync.dma_start(out=at_sb[:, 0:128], in_=a_t[:, 0:128])
            sl0v = bass.ts(0, GRP)
            sl0w = bass.ts(NPAIR, GRP)
            nc.scalar.dma_start(out=bt_sb[:, sl0v], in_=b_t[:, sl0v])
            nc.scalar.dma_start(out=bt_sb[:, sl0w], in_=b_t[:, sl0w])
            nc.sync.dma_start(out=at_sb[:, 128:], in_=a_t[:, 128:])
            for p in range(1, NPAIR):
                for half in (0, 1):
                    sl = bass.ts(half * NPAIR + p, GRP)
                    nc.sync.dma_start(out=bt_sb[:, sl], in_=b_t[:, sl])

            for p in range(NPAIR):
                vcols = p * GRP            # V-group column base
                wcols = 8 * GRP + p * GRP  # W-group column base
                for rt in range(ROWTILES):
                    lhsT = at_sb[:, bass.ts(rt, 128)]
                    pv = vpp.tile([128, GRP], dt.float32)
                    pw = spp.tile([128, GRP], dt.float32)
                    for j in range(GRP // BANK):
                        nc.tensor.matmul(
                            pv[:, bass.ts(j, BANK)],
                            lhsT,
                            bt_sb[:, bass.ts(vcols // BANK + j, BANK)],
                            start=True,
                            stop=True,
                        )
                    for j in range(GRP // BANK):
                        nc.tensor.matmul(
                            pw[:, bass.ts(j, BANK)],
                            lhsT,
                            bt_sb[:, bass.ts(wcols // BANK + j, BANK)],
                            start=True,
                            stop=True,
                        )
                    if p < NPAIR - 1:
                        nc.vector.tensor_reduce(
                            vt_sb[:, rt, p * (GRP // WIN) : (p + 1) * (GRP // WIN)],
                            pv[:].rearrange("p (c w) -> p c w", w=WIN),
                            axis=AX,
                            op=MAX,
                        )
                        slab = spool.tile([128, GRP], dt.float16)
                        nc.scalar.copy(slab[:], pw[:])
                        # sync has plenty of issue slack; Pool-issued SWDGE
                        # DMAs would add a ~6.4 us DGE drain at kernel exit
                        nc.sync.dma_start(
                            out=wraw[rt][:, bass.ts(p, GRP)], in_=slab[:]
                        )
                    else:
                        # last V-group is split VQ/(GRP-VQ) between the
                        # engines to balance DVE vs ScalarE load
                        nc.vector.tensor_reduce(
                            vt_sb[:, rt, 7 * (GRP // WIN) : NVCH],
                            pv[:, 0:VQ].rearrange("p (c w) -> p c w", w=WIN),
                            axis=AX,
                            op=MAX,
                        )
                        # rt's V-chunks are complete; ship before the
                        # evac chain so the issue isn't queued behind it
                        nc.sync.dma_start(
                            out=vout[rt][:], in_=vt_sb[:, rt, :]
                        )
                        slab7 = spool.tile([128, GRP + GRP - VQ], dt.float16)
                        nc.scalar.copy(slab7[:, GRP:], pv[:, VQ:GRP])
                        nc.scalar.copy(slab7[:, 0:GRP], pw[:])
                        # wraw cols [7168:8448) = positions [7168:8192) +
                        # the V-group tail [7936:8192)
                        nc.sync.dma_start(
                            out=wraw[rt][:, 7 * GRP : NW], in_=slab7[:]
                        )

    nc.compile()
    return nc


def _get_nc():
    if "nc" not in _CACHE:
        _CACHE["nc"] = _build_nc()
    return _CACHE["nc"]


def _prep(x_pred, x_target):
    """Host-side layout: targets sorted by b2 (identity chunk layout)."""
    import ml_dtypes

    b2 = np.einsum(
        "ij,ij->i", x_target.astype(np.float64), x_target.astype(np.float64)
    )
    order = np.argsort(b2, kind="stable")
    a_t = np.ascontiguousarray(2.0 * x_pred.T).astype(ml_dtypes.bfloat16)
    b_t = np.ascontiguousarray(x_target[order].T).astype(ml_dtypes.bfloat16)
    return a_t, b_t, b2, order


def _host_finish(x_pred, x_target, vo, wr, b2, order, k):
    n = x_pred.shape[0]
    a64 = x_pred.astype(np.float64)
    b64 = x_target.astype(np.float64)
    a2 = np.einsum("ij,ij->i", a64, a64)

    # V family: 496 width-16 chunks (consecutive ranks; first = min b2)
    b2min_v = b2[order[np.arange(NVCH) * WIN]].astype(np.float32)
    chv = np.argpartition(-(vo - b2min_v[None, :]), CV, axis=1)[:, :CV]
    tid_v = order[:V_END].reshape(NVCH, WIN)[chv].reshape(n, CV * WIN)

    # W: width-1 near-exact (2ab in fp16); d2 = a2 + b2 - 2ab
    # wraw cols [0:8192) = positions [8192:16384); cols [8192:8448) =
    # the V-group tail positions [7936:8192)
    w_ids = np.concatenate([order[8192:], order[V_END:8192]])
    d2_w = (
        a2[:, None].astype(np.float32)
        + b2[w_ids].astype(np.float32)[None, :]
        - wr
    )
    chw = np.argpartition(d2_w, KW, axis=1)[:, :KW]
    tid_w = w_ids[chw]

    tids = np.concatenate([tid_v, tid_w], axis=1)
    vals = np.empty((n, k))
    B = 2048
    for s in range(0, n, B):
        t = tids[s : s + B]
        dots = np.einsum("rd,rcd->rc", a64[s : s + B], b64[t], optimize=True)
        d2 = a2[s : s + B, None] + b2[t] - 2.0 * dots
        vals[s : s + B] = np.partition(d2, k - 1, axis=1)[:, :k]
    d = np.sqrt(np.maximum(vals, 0.0))
    return np.float32(np.maximum(d - HINGE, 0.0).mean(dtype=np.float64))


def _host_exact(x_pred, x_target, k):
    """Exact fallback (never expected in practice)."""
    a = x_pred.astype(np.float32)
    b = x_target.astype(np.float32)
    a2 = np.sum(a * a, axis=1)[:, None]
    b2 = np.sum(b * b, axis=1)[None, :]
    out = np.empty((a.shape[0], k), np.float64)
    B = 1024
    for s in range(0, a.shape[0], B):
        d2 = a2[s : s + B] + b2 - 2.0 * (a[s : s + B] @ b.T)
        out[s : s + B] = np.partition(d2, k - 1, axis=1)[:, :k].astype(np.float64)
    d = np.sqrt(np.maximum(out, 0.0))
    return np.float32(np.maximum(d - HINGE, 0.0).mean(dtype=np.float64))


def kernel(x_pred, x_target, top_k=5, _want_results=False):
    from concourse.bass_utils import run_bass_kernel_spmd

    x_pred = np.asarray(x_pred, dtype=np.float32)
    x_target = np.asarray(x_target, dtype=np.float32)
    k = int(top_k)
    if (
        k > KW
        or x_pred.shape != (N_PRED, DIM)
        or x_target.shape != (N_TGT, DIM)
        or not np.isfinite(x_pred).all()
        or not np.isfinite(x_target).all()
        or float(np.abs(x_pred).max()) * float(np.abs(x_target).max()) * DIM
        > 2.0e4
    ):
        return _host_exact(x_pred, x_target, k)

    nc = _get_nc()
    a_t_full, b_t, b2, order = _prep(x_pred, x_target)

    in_maps = []
    for c in range(N_CORES):
        in_maps.append(
            {
                "a_t": np.ascontiguousarray(
                    a_t_full[:, c * ROWS_PER_CORE : (c + 1) * ROWS_PER_CORE]
                ),
                "b_t": b_t,
            }
        )

    try:
        res = run_bass_kernel_spmd(nc, in_maps, list(range(N_CORES)))
    except Exception:
        # the NeuronCores occasionally come up wedged from a previous
        # process; the failed attempt resets them and a retry succeeds
        res = run_bass_kernel_spmd(nc, in_maps, list(range(N_CORES)))
    vo = np.concatenate(
        [
            res.results[c]["vout"].reshape(ROWS_PER_CORE, NVCH)
            for c in range(N_CORES)
        ],
        axis=0,
    ).astype(np.float32)
    wr = np.concatenate(
        [
            res.results[c]["wraw"].reshape(ROWS_PER_CORE, NW)
            for c in range(N_CORES)
        ],
        axis=0,
    ).astype(np.float32)
    out = _host_finish(x_pred, x_target, vo, wr, b2, order, k)
    if _want_results:
        return out, res
    return out


# revision 22
# speedup vs baseline: 1.0656x; 1.0656x over previous
"""Trainium2 Bass kernel for nn_DensityLoss (retrieval kNN hinge loss).

Computes mean(relu(topk_smallest_dist(x_pred, x_target, k) - 1.0)).

Strategy (8 NeuronCores, SPMD, x_pred rows sharded; 1024 rows/core):
  Every one of the 16384 dot products per row must be drained out of PSUM
  by exactly one compute-engine read (DMA and Pool cannot touch PSUM).
  The drain is split between the DVE and ScalarE, each with its own
  2-deep PSUM tile rotation so both stream independently:

  - V region (positions    0.. 8191): DVE tensor_reduce(axis=X) over a
    [128, 64, 16] PSUM view drains AND folds a 1024-group in one op ->
    512 width-16 chunk maxima per row.
  - W region (positions 8192..16383): ScalarE evacuates fp32->fp16 and
    the slab is shipped raw -> 8192 width-1 near-exact scores per row.

  The kernel runs group-PAIR-major (outer: 8 (V,W) group pairs; inner:
  8 rowtiles), so compute starts as soon as the first b_t slices land
  instead of waiting for the whole 4 MiB load, and the TensorE stays
  continuously busy (p-state).

  Host: targets are b2-sorted (pure sort, no permutation needed: chunk =
  16 consecutive ranks). Chunk score = chunk-max(2ab) - min b2; top-CV
  V-chunks + top-KW W-columns are rescored exactly in float64 -> top-k
  -> hinge -> mean.
"""

import numpy as np

N_CORES = 8
N_PRED = 8192
N_TGT = 16384
DIM = 128
ROWS_PER_CORE = N_PRED // N_CORES  # 1024
ROWTILES = ROWS_PER_CORE // 128    # 8
BANK = 512                         # fp32 PSUM bank, matmul max N
GRP = 1024                         # drain group width (2 PSUM banks)
NPAIR = 8                          # (V, W) group pairs per rowtile
WIN = 16                           # V-chunk width
VQ = 768                           # of the last V-group, DVE reduces
                                   # [0:VQ]; ScalarE evacs [VQ:1024]
                                   # (engine balance: DVE is the pacer)
V_END = 7 * GRP + VQ               # V region positions [0, 7936)
NVCH = V_END // WIN                # 496 V-chunks
NW = 8192 + (GRP - VQ)             # 8448 width-1 raw columns
CV = 12                            # top chunks, V family (width 16)
KW = 16                            # raw width-1 candidates
HINGE = 1.0

_CACHE = {}


def _build_nc():
    import concourse.bacc as bacc
    import concourse.bass as bass
    import concourse.mybir as mybir
    import concourse.tile as tile

    dt = mybir.dt
    nc = bacc.Bacc(
        "TRN2",
        target_bir_lowering=False,
        debug=False,
        num_devices=N_CORES,
    )
    a_t = nc.dram_tensor("a_t", [DIM, ROWS_PER_CORE], dt.bfloat16, kind="ExternalInput")
    b_t = nc.dram_tensor("b_t", [DIM, N_TGT], dt.bfloat16, kind="ExternalInput")
    vout = nc.dram_tensor(
        "vout", [ROWTILES, 128, NVCH], dt.float16, kind="ExternalOutput"
    )
    wraw = nc.dram_tensor(
        "wraw", [ROWTILES, 128, NW], dt.float16, kind="ExternalOutput"
    )

    AX = mybir.AxisListType.X
    MAX = mybir.AluOpType.max

    with tile.TileContext(nc) as tc:
        with (
            tc.tile_pool(name="const", bufs=1) as cpool,
            tc.tile_pool(name="vp", bufs=2, space="PSUM") as vpp,
            tc.tile_pool(name="sp", bufs=2, space="PSUM") as spp,
            tc.tile_pool(name="slab", bufs=6) as spool,
        ):
            bt_sb = cpool.tile([DIM, N_TGT], dt.bfloat16)
            at_sb = cpool.tile([DIM, ROWS_PER_CORE], dt.bfloat16)
            # per-rowtile V-chunk accumulators, DMA'd out at the end
            vt_sb = cpool.tile([128, ROWTILES, NVCH], dt.float16)

            # Startup-critical inputs issue from TWO idle sequencers in
            # parallel (each HWDGE issue costs ~600-670 ns; serializing
            # them on sync alone delays the first matmuls by ~2-3 us):
            # sync: a_t, scalar: pair-0's two b_t slices, then sync
            # streams the rest in first-use order. (Only SP/Activation
            # can initiate HWDGE DMAs.)
            nc.sync.dma_start(out=at_sb[:, 0:128], in_=a_t[:, 0:128])
            sl0v = bass.ts(0, GRP)
            sl0w = bass.ts(NPAIR, GRP)
            nc.scalar.dma_start(out=bt_sb[:, sl0v], in_=b_t[:, sl0v])
            nc.scalar.dma_start(out=bt_sb[:, sl0w], in_=b_t[:, sl0w])
            nc.sync.dma_start(out=at_sb[:, 128:], in_=a_t[:, 128:])
            for p in range(1, NPAIR):
                for half in (0, 1):
                    sl = bass.ts(half * NPAIR + p, GRP)
                    nc.sync.dma_start(out=bt_sb[:, sl], in_=b_t[:, sl])

            for p in range(NPAIR):
                vcols = p * GRP            # V-group column base
                wcols = 8 * GRP + p * GRP  # W-group column base
                for rt in range(ROWTILES):
                    lhsT = at_sb[:, bass.ts(rt, 128)]
                    pv = vpp.tile([128, GRP], dt.float32)
                    pw = spp.tile([128, GRP], dt.float32)
                    for j in range(GRP // BANK):
                        nc.tensor.matmul(
                            pv[:, bass.ts(j, BANK)],
                            lhsT,
                            bt_sb[:, bass.ts(vcols // BANK + j, BANK)],
                            start=True,
                            stop=True,
                        )
                    for j in range(GRP // BANK):
                        nc.tensor.matmul(
                            pw[:, bass.ts(j, BANK)],
                            lhsT,
                            bt_sb[:, bass.ts(wcols // BANK + j, BANK)],
                            start=True,
                            stop=True,
                        )
                    if p < NPAIR - 1:
                        nc.vector.tensor_reduce(
                            vt_sb[:, rt, p * (GRP // WIN) : (p + 1) * (GRP // WIN)],
                            pv[:].rearrange("p (c w) -> p c w", w=WIN),
                            axis=AX,
                            op=MAX,
                        )
                        slab = spool.tile([128, GRP], dt.float16)
                        nc.scalar.copy(slab[:], pw[:])
                        # sync has plenty of issue slack; Pool-issued SWDGE
                        # DMAs would add a ~6.4 us DGE drain at kernel exit
                        nc.sync.dma_start(
                            out=wraw[rt][:, bass.ts(p, GRP)], in_=slab[:]
                        )
                    else:
                        # last V-group is split VQ/(GRP-VQ) between the
                        # engines to balance DVE vs ScalarE load
                        nc.vector.tensor_reduce(
                            vt_sb[:, rt, 7 * (GRP // WIN) : NVCH],
                            pv[:, 0:VQ].rearrange("p (c w) -> p c w", w=WIN),
                            axis=AX,
                            op=MAX,
                        )
                        # rt's V-chunks are complete; ship before the
                        # evac chain so the issue isn't queued behind it
                        nc.sync.dma_start(
                            out=vout[rt][:], in_=vt_sb[:, rt, :]
                        )
                        slab7 = spool.tile([128, GRP + GRP - VQ], dt.float16)
                        nc.scalar.copy(slab7[:, GRP:], pv[:, VQ:GRP])
                        nc.scalar.copy(slab7[:, 0:GRP], pw[:])
                        # wraw cols [7168:8448) = positions [7168:8192) +
                        # the V-group tail [7936:8192)
                        nc.sync.dma_start(
                            out=wraw[rt][:, 7 * GRP : NW], in_=slab7[:]
                        )

    nc.compile()
    return nc


def _get_nc():
    if "nc" not in _CACHE:
        _CACHE["nc"] = _build_nc()
    return _CACHE["nc"]


def _prep(x_pred, x_target):
    """Host-side layout: targets sorted by b2 (identity chunk layout)."""
    import ml_dtypes

    b2 = np.einsum(
        "ij,ij->i", x_target.astype(np.float64), x_target.astype(np.float64)
    )
    order = np.argsort(b2, kind="stable")
    a_t = np.ascontiguousarray(2.0 * x_pred.T).astype(ml_dtypes.bfloat16)
    b_t = np.ascontiguousarray(x_target[order].T).astype(ml_dtypes.bfloat16)
    return a_t, b_t, b2, order


def _host_finish(x_pred, x_target, vo, wr, b2, order, k):
    n = x_pred.shape[0]
    a64 = x_pred.astype(np.float64)
    b64 = x_target.astype(np.float64)
    a2 = np.einsum("ij,ij->i", a64, a64)

    # V family: 496 width-16 chunks (consecutive ranks; first = min b2)
    b2min_v = b2[order[np.arange(NVCH) * WIN]].astype(np.float32)
    chv = np.argpartition(-(vo - b2min_v[None, :]), CV, axis=1)[:, :CV]
    tid_v = order[:V_END].reshape(NVCH, WIN)[chv].reshape(n, CV * WIN)

    # W: width-1 near-exact (2ab in fp16); d2 = a2 + b2 - 2ab
    # wraw cols [0:8192) = positions [8192:16384); cols [8192:8448) =
    # the V-group tail positions [7936:8192)
    w_ids = np.concatenate([order[8192:], order[V_END:8192]])
    d2_w = (
        a2[:, None].astype(np.float32)
        + b2[w_ids].astype(np.float32)[None, :]
        - wr
    )
    chw = np.argpartition(d2_w, KW, axis=1)[:, :KW]
    tid_w = w_ids[chw]

    tids = np.concatenate([tid_v, tid_w], axis=1)
    vals = np.empty((n, k))
    B = 2048
    for s in range(0, n, B):
        t = tids[s : s + B]
        dots = np.einsum("rd,rcd->rc", a64[s : s + B], b64[t], optimize=True)
        d2 = a2[s : s + B, None] + b2[t] - 2.0 * dots
        vals[s : s + B] = np.partition(d2, k - 1, axis=1)[:, :k]
    d = np.sqrt(np.maximum(vals, 0.0))
    return np.float32(np.maximum(d - HINGE, 0.0).mean(dtype=np.float64))


def _host_exact(x_pred, x_target, k):
    """Exact fallback (never expected in practice)."""
    a = x_pred.astype(np.float32)
    b = x_target.astype(np.float32)
    a2 = np.sum(a * a, axis=1)[:, None]
    b2 = np.sum(b * b, axis=1)[None, :]
    out = np.empty((a.shape[0], k), np.float64)
    B = 1024
    for s in range(0, a.shape[0], B):
        d2 = a2[s : s + B] + b2 - 2.0 * (a[s : s + B] @ b.T)
        out[s : s + B] = np.partition(d2, k - 1, axis=1)[:, :k].astype(np.float64)
    d = np.sqrt(np.maximum(out, 0.0))
    return np.float32(np.maximum(d - HINGE, 0.0).mean(dtype=np.float64))


def kernel(x_pred, x_target, top_k=5, _want_results=False):
    from concourse.bass_utils import run_bass_kernel_spmd

    x_pred = np.asarray(x_pred, dtype=np.float32)
    x_target = np.asarray(x_target, dtype=np.float32)
    k = int(top_k)
    if (
        k > KW
        or x_pred.shape != (N_PRED, DIM)
        or x_target.shape != (N_TGT, DIM)
        or not np.isfinite(x_pred).all()
        or not np.isfinite(x_target).all()
        or float(np.abs(x_pred).max()) * float(np.abs(x_target).max()) * DIM
        > 2.0e4
    ):
        return _host_exact(x_pred, x_target, k)

    nc = _get_nc()
    a_t_full, b_t, b2, order = _prep(x_pred, x_target)

    in_maps = []
    for c in range(N_CORES):
        in_maps.append(
            {
                "a_t": np.ascontiguousarray(
                    a_t_full[:, c * ROWS_PER_CORE : (c + 1) * ROWS_PER_CORE]
                ),
                "b_t": b_t,
            }
        )

    try:
        res = run_bass_kernel_spmd(nc, in_maps, list(range(N_CORES)))
    except Exception:
        # the NeuronCores occasionally come up wedged from a previous
        # process; the failed attempt resets them and a retry succeeds
        res = run_bass_kernel_spmd(nc, in_maps, list(range(N_CORES)))
    vo = np.concatenate(
        [
            res.results[c]["vout"].reshape(ROWS_PER_CORE, NVCH)
            for c in range(N_CORES)
        ],
        axis=0,
    ).astype(np.float32)
    wr = np.concatenate(
        [
            res.results[c]["wraw"].reshape(ROWS_PER_CORE, NW)
            for c in range(N_CORES)
        ],
        axis=0,
    ).astype(np.float32)
    out = _host_finish(x_pred, x_target, vo, wr, b2, order, k)
    if _want_results:
        return out, res
    return out
